# revision 6
# baseline (speedup 1.0000x reference)
"""DeMBR multi-behavior LightGCN kernel for Trainium2 (8 NeuronCores).

Strategy (per dense behavior, each [N,N] relation matrix R):
  - Host pre-casts R to bf16. Row-shard across 8 cores (512 users each).
  - Each core loads its shard twice from HBM, both as plain HWDGE DMAs on
    separate FIFOs: natural layout (streamed in 8 column chunks) and
    transposed layout (DMA-xbar transpose-load straight from DRAM).
  - All propagation products are PE matmuls with the big matrix as the
    moving operand (512-wide free dim):
      C2:    u1_un.T (+deg_u via ones column)  = [i0|1].T @ R^T-shard
      C1+C4: z.T / w.T packed                  = [u0|u0+u1].T @ R-shard
      C3:    u2_un.T                           = i1.T @ R^T-shard
  - One [64, 4096] fp32 AllReduce per behavior combines z = R^T u0 across
    cores (needed on-device for the layer-2 user side). The item-side
    output i_acc = (z + R^T u1) / (2 deg_i) is assembled on the host from
    the AllReduced z plus per-core w = R^T(u0+u1) partials.
  - deg_i (a column-sum of the input matrix) is computed on the host in one
    pass; its reciprocal is fed to the device for the i1 normalization.
    deg_u comes free as the ones-column of the C2 matmul.
  - All-ones matrices (the virtual-behavior M's at init) are detected on
    the host and computed analytically (ones @ X is a broadcast column sum).

kernel(**inputs) takes the full unsharded inputs and returns [14, 4096, 64].
"""

import os
import numpy as np
import ml_dtypes

EPS = 1e-8
N, D = 4096, 64
P = 128
NCORES = 8
ULOC = N // NCORES          # 512 users per core
NU = ULOC // P              # 4 user chunks
NI = N // P                 # 32 item chunks
CH = 512                    # moving free-dim chunk
NCH = N // CH               # 8 chunks for the user-side contractions

_BF16 = ml_dtypes.bfloat16


# --------------------------------------------------------------------------
# device program
# --------------------------------------------------------------------------

def build_program(nb):
    """Build + bacc-compile the SPMD program for `nb` dense behaviors."""
    import concourse.bass as bass  # noqa: F401  (registers types)
    import concourse.mybir as mybir
    import concourse.tile as tile
    from concourse import bacc
    from concourse.masks import make_identity

    f32, bf16 = mybir.dt.float32, mybir.dt.bfloat16
    ALU = mybir.AluOpType
    CPY = mybir.ActivationFunctionType.Copy

    nc = bacc.Bacc("TRN2", target_bir_lowering=False, debug=False,
                   num_devices=NCORES)

    R_in = [nc.dram_tensor(f"R{b}", [ULOC, N], bf16, kind="ExternalInput")
            for b in range(nb)]
    ri_in = [nc.dram_tensor(f"ri{b}", [P, NI], f32, kind="ExternalInput")
             for b in range(nb)]
    i0s_in = nc.dram_tensor("i0s", [P, NI, D + 1], bf16, kind="ExternalInput")
    u0s_in = nc.dram_tensor("u0s", [P, NU, D], bf16, kind="ExternalInput")
    uacc_out = [nc.dram_tensor(f"uacc{b}", [P, NU, D], f32, kind="ExternalOutput")
                for b in range(nb)]
    w_out = [nc.dram_tensor(f"wT{b}", [D, N], f32, kind="ExternalOutput")
             for b in range(nb)]
    z_r_out = [nc.dram_tensor(f"zr{b}", [D, N], f32, kind="ExternalOutput")
               for b in range(nb)]

    rg = [list(range(NCORES))]

    with tile.TileContext(nc) as tc:
        with (
            tc.tile_pool(name="big", bufs=3) as pbig,
            tc.tile_pool(name="chunk", bufs=4) as pchunk,
            tc.tile_pool(name="small", bufs=2) as psm,
            tc.tile_pool(name="one", bufs=1) as pone,
            tc.tile_pool(name="mm", bufs=2, space="PSUM") as pmm,
            tc.tile_pool(name="mm14", bufs=3, space="PSUM") as pmm14,
            tc.tile_pool(name="tr", bufs=1, space="PSUM") as ptr,
            tc.tile_pool(name="dram", bufs=2, space="DRAM") as pdr,
        ):
            ident = pone.tile([P, P], f32)
            make_identity(nc, ident[:])
            i0s = pone.tile([P, NI, D + 1], bf16)
            nc.sync.dma_start(out=i0s[:], in_=i0s_in[:])
            u0s = pone.tile([P, NU, D], bf16)
            nc.sync.dma_start(out=u0s[:], in_=u0s_in[:])

            for b in range(nb):
                # ---- R^T copy via xbar transpose-load straight from DRAM
                #      (scalar-engine HWDGE FIFO; independent of everything)
                At = pbig.tile([P, NI, ULOC], bf16, tag="At", name=f"At{b}")
                for uc in range(NU):
                    nc.scalar.dma_start_transpose(
                        out=At[:, :, uc * P:(uc + 1) * P],
                        in_=R_in[b][uc * P:(uc + 1) * P, :])
                ris = psm.tile([P, NI], f32, tag="ris", name=f"ris{b}")
                nc.sync.dma_start(out=ris[:], in_=ri_in[b].ap())

                # ---- C2: psum [65, 512] = [i0|1].T @ R^T  (accum over items)
                P2 = pmm.tile([D + 1, CH], f32, tag="PC", name=f"P2_{b}")
                for ic in range(NI):
                    nc.tensor.matmul(P2[:], i0s[:, ic, :], At[:, ic, :],
                                     start=(ic == 0), stop=(ic == NI - 1))
                S2 = psm.tile([D + 1, CH], f32, tag="S2", name=f"S2_{b}")
                nc.vector.tensor_copy(out=S2[:], in_=P2[:])
                PT2 = ptr.tile([P, NU, D + 1], f32, tag="PT2", name=f"PT2_{b}")
                for uc in range(NU):
                    nc.tensor.transpose(PT2[:, uc, :],
                                        S2[:, uc * P:(uc + 1) * P],
                                        ident[0:D + 1, 0:D + 1])
                rut = psm.tile([P, NU, 1], f32, tag="rut", name=f"rut{b}")
                nc.vector.tensor_scalar_add(out=rut[:], in0=PT2[:, :, D:D + 1],
                                            scalar1=EPS)
                ru = psm.tile([P, NU, 1], f32, tag="ru", name=f"ru{b}")
                nc.vector.reciprocal(out=ru[:], in_=rut[:])
                u1f = psm.tile([P, NU, D], f32, tag="u1f", name=f"u1f{b}")
                for uc in range(NU):
                    nc.scalar.activation(out=u1f[:, uc, :], in_=PT2[:, uc, 0:D],
                                         func=CPY, scale=ru[:, uc, :])
                u1b = psm.tile([P, NU, D], bf16, tag="u1b", name=f"u1b{b}")
                nc.vector.tensor_copy(out=u1b[:], in_=u1f[:])
                L = psm.tile([P, NU, 2 * D], bf16, tag="L", name=f"L{b}")
                nc.vector.tensor_copy(out=L[:, :, 0:D], in_=u0s[:])
                nc.vector.tensor_copy(out=L[:, :, D:2 * D], in_=u1b[:])

                # ---- C1+C4 packed: [u0 | u1].T @ R-shard -> z.T | z2.T
                #      R-shard streamed from DRAM in [128, 4, 512] chunks
                zT = pone.tile([D, N], f32, tag="zT", name=f"zT{b}")
                wT = pone.tile([D, N], f32, tag="wT", name=f"wT{b}")
                for n in range(NCH):
                    Ac = pchunk.tile([P, NU, CH], bf16, tag="Ac",
                                     name=f"Ac{b}_{n}")
                    nc.sync.dma_start(
                        out=Ac[:],
                        in_=R_in[b].ap().rearrange(
                            "(uc p) n -> p uc n", p=P)[:, :,
                                                       n * CH:(n + 1) * CH])
                    P14 = pmm14.tile([P, CH], f32, tag="P14",
                                     name=f"P14_{b}_{n}")
                    for uc in range(NU):
                        nc.tensor.matmul(P14[:], L[:, uc, :], Ac[:, uc, :],
                                         start=(uc == 0), stop=(uc == NU - 1))
                    nc.vector.tensor_copy(out=zT[:, n * CH:(n + 1) * CH],
                                          in_=P14[0:D, :])
                    nc.vector.tensor_copy(out=wT[:, n * CH:(n + 1) * CH],
                                          in_=P14[D:2 * D, :])

                # ---- AllReduce z
                z_in = pdr.tile([D, N], f32, tag="z_in", name=f"z_in{b}")
                nc.scalar.dma_start(out=z_in[:], in_=zT[:])
                z_out = pdr.tile([D, N], f32, tag="z_out",
                                 name=f"z_out{b}", addr_space="Shared")
                nc.gpsimd.collective_compute(
                    "AllReduce", ALU.add, replica_groups=rg,
                    ins=[z_in.opt()], outs=[z_out.opt()])
                nc.scalar.dma_start(out=z_r_out[b].ap(), in_=z_out[:])

                # ---- i1 = z * ri in natural layout (cast, xbar transpose,
                #      per-item scale on the scalar engine)
                zb = pone.tile([D, N], bf16, tag="zb", name=f"zb{b}")
                nc.gpsimd.dma_start(out=zb[:], in_=z_out[:])
                i1u = pone.tile([P, NI, D], bf16, tag="i1u", name=f"i1u{b}")
                nc.sync.dma_start_transpose(out=i1u[:], in_=zb[:])
                i1b = pone.tile([P, NI, D], bf16, tag="i1b", name=f"i1b{b}")
                for ic in range(NI):
                    nc.scalar.activation(out=i1b[:, ic, :], in_=i1u[:, ic, :],
                                         func=CPY, scale=ris[:, ic:ic + 1])

                # ---- C3: u2_un.T = i1.T @ R^T
                P3 = pmm.tile([D, CH], f32, tag="PC", name=f"P3_{b}")
                for ic in range(NI):
                    nc.tensor.matmul(P3[:], i1b[:, ic, :], At[:, ic, :],
                                     start=(ic == 0), stop=(ic == NI - 1))
                S3 = psm.tile([D, CH], f32, tag="S3", name=f"S3_{b}")
                nc.vector.tensor_copy(out=S3[:], in_=P3[:])
                PT3 = ptr.tile([P, NU, D], f32, tag="PT3", name=f"PT3_{b}")
                for uc in range(NU):
                    nc.tensor.transpose(PT3[:, uc, :],
                                        S3[:, uc * P:(uc + 1) * P],
                                        ident[0:D, 0:D])
                u2f = psm.tile([P, NU, D], f32, tag="u2f", name=f"u2f{b}")
                for uc in range(NU):
                    nc.scalar.activation(out=u2f[:, uc, :], in_=PT3[:, uc, :],
                                         func=CPY, scale=ru[:, uc, :])
                uacc = psm.tile([P, NU, D], f32, tag="uacc", name=f"uacc{b}")
                nc.vector.tensor_add(out=uacc[:], in0=u2f[:], in1=u1f[:])

                nc.sync.dma_start(out=uacc_out[b].ap(), in_=uacc[:])
                nc.sync.dma_start(out=w_out[b].ap(), in_=wT[:])

    nc.compile()
    return nc


# --------------------------------------------------------------------------
# host-side helpers
# --------------------------------------------------------------------------

def _swz_items(x):
    """[4096, C] -> [128, 32, C] with item = ic*128 + p."""
    return np.ascontiguousarray(x.reshape(NI, P, x.shape[1]).transpose(1, 0, 2))


def _swz_users(x):
    """[512, C] -> [128, 4, C] with user = uc*128 + p."""
    return np.ascontiguousarray(x.reshape(NU, P, x.shape[1]).transpose(1, 0, 2))


def prep_in_maps(dense_mats, u0, i0):
    """dense_mats: list of (R_bf16 [N,N], ri_nat [128, 32] f32)."""
    i0_aug = np.concatenate(
        [i0.astype(_BF16), np.ones((N, 1), _BF16)], axis=1)
    i0s = _swz_items(i0_aug)
    in_maps = []
    for k in range(NCORES):
        m = {"i0s": i0s,
             "u0s": _swz_users(u0[k * ULOC:(k + 1) * ULOC].astype(_BF16))}
        for b, (Rb, ri_nat) in enumerate(dense_mats):
            m[f"R{b}"] = np.ascontiguousarray(Rb[k * ULOC:(k + 1) * ULOC, :])
            m[f"ri{b}"] = ri_nat
        in_maps.append(m)
    return in_maps


def host_prep_behavior(R):
    """Cast to bf16 + compute item-degree reciprocal (natural layout)."""
    Rb = R.astype(_BF16)
    deg = R.sum(axis=0, dtype=np.float64)
    ri_vec = (1.0 / (deg + EPS)).astype(np.float32)
    ri_nat = np.ascontiguousarray(ri_vec.reshape(NI, P).T)
    return Rb, ri_nat, deg.astype(np.float32)


def assemble_dense(results, degs, nb):
    """Per-behavior (u_acc [N,D], i_acc [N,D]) from per-core outputs."""
    out = []
    for b in range(nb):
        u = np.concatenate(
            [results[k][f"uacc{b}"].transpose(1, 0, 2).reshape(ULOC, D)
             for k in range(NCORES)], axis=0) * np.float32(0.5)
        w = np.sum([results[k][f"wT{b}"] for k in range(NCORES)], axis=0,
                   dtype=np.float32)
        z = results[0][f"zr{b}"]
        i_acc = ((z + w) * np.float32(0.5)
                 / (degs[b] + np.float32(EPS))[None, :]).T
        out.append((np.ascontiguousarray(u, dtype=np.float32),
                    np.ascontiguousarray(i_acc, dtype=np.float32)))
    return out


def ones_behavior(u0, i0):
    """Analytic LightGCN-2-layer outputs when R is all-ones [N, N]."""
    s_i = i0.astype(np.float64).sum(axis=0)
    s_u = u0.astype(np.float64).sum(axis=0)
    d = N + EPS
    u_row = (s_i / d + s_u * N / (d * d)) * 0.5
    i_row = (s_u / d + s_i * N / (d * d)) * 0.5
    u = np.broadcast_to(u_row.astype(np.float32), (N, D)).copy()
    it = np.broadcast_to(i_row.astype(np.float32), (N, D)).copy()
    return u, it


# --------------------------------------------------------------------------
# cached device runner (compile once per behavior-count, run many)
# --------------------------------------------------------------------------

_RUNNERS = {}


class _Runner:
    def __init__(self, nb):
        self.nb = nb
        self.nc = build_program(nb)
        self._jitted = None
        self._meta = None

    def _prep_jit(self):
        import jax
        import numpy as _np
        from jax.sharding import Mesh, PartitionSpec
        from jax.experimental.shard_map import shard_map
        from concourse import bass2jax
        from concourse.bass2jax import _bass_exec_p, partition_id_tensor
        import concourse.mybir as mybir

        bass2jax.install_neuronx_cc_hook()
        nc = self.nc
        partition_name = (nc.partition_id_tensor.name
                          if nc.partition_id_tensor else None)
        in_names, out_names, out_avals, zero_shapes = [], [], [], []
        for alloc in nc.m.functions[0].allocations:
            if not isinstance(alloc, mybir.MemoryLocationSet):
                continue
            name = alloc.memorylocations[0].name
            if alloc.kind == "ExternalInput":
                if name != partition_name:
                    in_names.append(name)
            elif alloc.kind == "ExternalOutput":
                shape = tuple(alloc.tensor_shape)
                dtype = mybir.dt.np(alloc.dtype)
                out_names.append(name)
                out_avals.append(jax.core.ShapedArray(shape, dtype))
                zero_shapes.append((shape, dtype))
        n_params = len(in_names)
        full_in_names = list(in_names) + list(out_names)
        if partition_name is not None:
            full_in_names.append(partition_name)

        def _body(*args):
            operands = list(args)
            if partition_name is not None:
                operands.append(partition_id_tensor())
            outs = _bass_exec_p.bind(
                *operands,
                out_avals=tuple(out_avals),
                in_names=tuple(full_in_names),
                out_names=tuple(out_names),
                lowering_input_output_aliases=(),
                sim_require_finite=True,
                sim_require_nnan=True,
                nc=nc,
            )
            return tuple(outs)

        devices = jax.devices()[:NCORES]
        mesh = Mesh(_np.asarray(devices), ("core",))
        n_outs = len(out_names)
        in_specs = (PartitionSpec("core"),) * (n_params + n_outs)
        out_specs = (PartitionSpec("core"),) * n_outs
        donate = tuple(range(n_params, n_params + n_outs))
        self._jitted = jax.jit(
            shard_map(_body, mesh=mesh, in_specs=in_specs,
                      out_specs=out_specs, check_rep=False),
            donate_argnums=donate, keep_unused=True)
        self._meta = (in_names, out_names, out_avals, zero_shapes, n_params)

    def run(self, in_maps):
        if self._jitted is None:
            self._prep_jit()
        import numpy as _np
        in_names, out_names, out_avals, zero_shapes, n_params = self._meta
        concat_in = [
            _np.concatenate([_np.asarray(in_maps[c][nm]) for c in range(NCORES)],
                            axis=0)
            for nm in in_names]
        concat_zeros = [_np.zeros((NCORES * s[0], *s[1:]), dt)
                        for (s, dt) in zero_shapes]
        out_arrs = self._jitted(*concat_in, *concat_zeros)
        results = []
        for c in range(NCORES):
            results.append({
                nm: _np.asarray(out_arrs[i]).reshape(
                    NCORES, *out_avals[i].shape)[c]
                for i, nm in enumerate(out_names)})
        return results

    def run_traced(self, in_maps, tmpdir=None):
        """Run through run_bass_kernel_spmd with NTFF tracing (recompiles)."""
        _install_trace_shims()
        from concourse.bass_utils import run_bass_kernel_spmd
        return run_bass_kernel_spmd(self.nc, in_maps,
                                    core_ids=list(range(NCORES)),
                                    trace=True, tmpdir=tmpdir)


def _install_trace_shims():
    """This image's antenv lacks axon_hooks (the NTFF-hook registry) and has
    no artifact bucket; recreate the hook from the boot recipe and make
    artifact upload a local no-op."""
    import sys, types, importlib.util

    if "antenv.axon_hooks" not in sys.modules:
        mod = types.ModuleType("antenv.axon_hooks")
        mod._hook = None

        def set_axon_ntff_profile_hook(h):
            mod._hook = h

        def get_axon_ntff_profile_hook():
            return mod._hook

        mod.set_axon_ntff_profile_hook = set_axon_ntff_profile_hook
        mod.get_axon_ntff_profile_hook = get_axon_ntff_profile_hook
        import antenv
        sys.modules["antenv.axon_hooks"] = mod
        antenv.axon_hooks = mod

        spec = importlib.util.spec_from_file_location(
            "trn_boot_shim", "/root/.axon_site/trn_agent_boot/trn_boot.py")
        boot = importlib.util.module_from_spec(spec)
        spec.loader.exec_module(boot)
        hook = boot._ntff_profile_via_ctypes("/opt/axon/libaxon_pjrt.so")
        mod._hook = hook

    import concourse.bass_utils as bu
    if not getattr(bu.upload_artifacts, "_is_local_shim", False):
        def _local_upload(tmpdir):
            return tmpdir
        _local_upload._is_local_shim = True
        bu.upload_artifacts = _local_upload


def get_runner(nb):
    if nb not in _RUNNERS:
        _RUNNERS[nb] = _Runner(nb)
    return _RUNNERS[nb]


# --------------------------------------------------------------------------
# entry point
# --------------------------------------------------------------------------

def _is_ones(a):
    return a[0, 0] == 1.0 and bool(np.all(a == np.float32(1.0)))


def kernel(**inputs):
    inputs = {k: np.asarray(v) for k, v in inputs.items()}
    u0 = np.ascontiguousarray(inputs["user_embedding"], dtype=np.float32)
    i0 = np.ascontiguousarray(inputs["item_embedding"], dtype=np.float32)

    real_names = ["R_click", "R_fav", "R_cart", "R_buy"]
    virt_names = [("M_click", "add_click"), ("M_fav", "add_fav"),
                  ("M_cart", "add_cart")]
    mats = [np.asarray(inputs[n], dtype=np.float32) for n in real_names]
    mats += [np.asarray(inputs[m], dtype=np.float32) for m, _ in virt_names]

    dense_idx = [j for j, a in enumerate(mats) if not _is_ones(a)]
    per_behavior = [None] * 7

    if dense_idx:
        nb = len(dense_idx)
        runner = get_runner(nb)
        prepped = [host_prep_behavior(mats[j]) for j in dense_idx]
        in_maps = prep_in_maps([(p[0], p[1]) for p in prepped], u0, i0)
        results = runner.run(in_maps)
        dense = assemble_dense(results, [p[2] for p in prepped], nb)
        for pos, j in enumerate(dense_idx):
            per_behavior[j] = dense[pos]

    ones_cache = None
    for j, a in enumerate(mats):
        if per_behavior[j] is None:
            if ones_cache is None:
                ones_cache = ones_behavior(u0, i0)
            per_behavior[j] = ones_cache

    ur = [per_behavior[j][0] for j in range(4)]
    ir = [per_behavior[j][1] for j in range(4)]
    uv = [per_behavior[4 + j][0] + np.asarray(inputs[virt_names[j][1]],
                                              dtype=np.float32)
          for j in range(3)]
    iv = [per_behavior[4 + j][1] for j in range(3)]

    out = np.concatenate(
        [np.stack(ur), np.stack(ir), np.stack(uv), np.stack(iv)], axis=0)
    return np.ascontiguousarray(out, dtype=np.float32)


# revision 7
# speedup vs baseline: 1.4046x; 1.4046x over previous
"""DeMBR multi-behavior LightGCN kernel for Trainium2 (8 NeuronCores).

Strategy (per dense behavior, each [N,N] relation matrix R):
  - Host pre-casts R to bf16. Row-shard across 8 cores (512 users each).
  - Each core loads its shard twice from HBM, both as plain HWDGE DMAs on
    separate FIFOs: natural layout (streamed in 8 column chunks) and
    transposed layout (DMA-xbar transpose-load straight from DRAM).
  - All propagation products are PE matmuls with the big matrix as the
    moving operand (512-wide free dim):
      C2:    u1_un.T (+deg_u via ones column)  = [i0|1].T @ R^T-shard
      C1+C4: z.T / w.T packed                  = [u0|u0+u1].T @ R-shard
      C3:    u2_un.T                           = i1.T @ R^T-shard
  - One [64, 4096] fp32 AllReduce per behavior combines z = R^T u0 across
    cores (needed on-device for the layer-2 user side). The item-side
    output i_acc = (z + R^T u1) / (2 deg_i) is assembled on the host from
    the AllReduced z plus per-core w = R^T(u0+u1) partials.
  - deg_i (a column-sum of the input matrix) is computed on the host in one
    pass; its reciprocal is fed to the device for the i1 normalization.
    deg_u comes free as the ones-column of the C2 matmul.
  - All-ones matrices (the virtual-behavior M's at init) are detected on
    the host and computed analytically (ones @ X is a broadcast column sum).

kernel(**inputs) takes the full unsharded inputs and returns [14, 4096, 64].
"""

import os
import numpy as np
import ml_dtypes

EPS = 1e-8
N, D = 4096, 64
P = 128
NCORES = 8
ULOC = N // NCORES          # 512 users per core
NU = ULOC // P              # 4 user chunks
NI = N // P                 # 32 item chunks
CH = 512                    # moving free-dim chunk
NCH = N // CH               # 8 chunks for the user-side contractions

_BF16 = ml_dtypes.bfloat16


# --------------------------------------------------------------------------
# device program
# --------------------------------------------------------------------------

def build_program(nb):
    """Build + bacc-compile the SPMD program for `nb` dense behaviors."""
    import concourse.bass as bass  # noqa: F401  (registers types)
    import concourse.mybir as mybir
    import concourse.tile as tile
    from concourse import bacc
    from concourse.masks import make_identity

    f32, bf16 = mybir.dt.float32, mybir.dt.bfloat16
    ALU = mybir.AluOpType
    CPY = mybir.ActivationFunctionType.Copy

    nc = bacc.Bacc("TRN2", target_bir_lowering=False, debug=False,
                   num_devices=NCORES)

    R_in = [nc.dram_tensor(f"R{b}", [ULOC, N], bf16, kind="ExternalInput")
            for b in range(nb)]
    Rt_in = [nc.dram_tensor(f"Rt{b}", [N, ULOC], bf16, kind="ExternalInput")
             for b in range(nb)]
    ri_in = [nc.dram_tensor(f"ri{b}", [P, NI], f32, kind="ExternalInput")
             for b in range(nb)]
    i0s_in = nc.dram_tensor("i0s", [P, NI, D + 1], bf16, kind="ExternalInput")
    u0s_in = nc.dram_tensor("u0s", [P, NU, D], bf16, kind="ExternalInput")
    uacc_out = [nc.dram_tensor(f"uacc{b}", [P, NU, D], f32, kind="ExternalOutput")
                for b in range(nb)]
    w_out = [nc.dram_tensor(f"wT{b}", [D, N], f32, kind="ExternalOutput")
             for b in range(nb)]
    z_r_out = [nc.dram_tensor(f"zr{b}", [D, N], f32, kind="ExternalOutput")
               for b in range(nb)]

    rg = [list(range(NCORES))]

    with tile.TileContext(nc) as tc:
        with (
            tc.tile_pool(name="big", bufs=3) as pbig,
            tc.tile_pool(name="chunk", bufs=4) as pchunk,
            tc.tile_pool(name="small", bufs=2) as psm,
            tc.tile_pool(name="one", bufs=1) as pone,
            tc.tile_pool(name="mm", bufs=2, space="PSUM") as pmm,
            tc.tile_pool(name="mm14", bufs=3, space="PSUM") as pmm14,
            tc.tile_pool(name="tr", bufs=1, space="PSUM") as ptr,
            tc.tile_pool(name="dram", bufs=2, space="DRAM") as pdr,
        ):
            ident = pone.tile([P, P], f32)
            make_identity(nc, ident[:])
            i0s = pone.tile([P, NI, D + 1], bf16)
            nc.sync.dma_start(out=i0s[:], in_=i0s_in[:])
            u0s = pone.tile([P, NU, D], bf16)
            nc.sync.dma_start(out=u0s[:], in_=u0s_in[:])

            state = {}

            def front(b):
                # ---- R^T copy: plain strided load of the host-pretransposed
                #      shard (scalar-engine HWDGE FIFO)
                At = pbig.tile([P, NI, ULOC], bf16, tag="At", name=f"At{b}")
                nc.scalar.dma_start(
                    out=At[:],
                    in_=Rt_in[b].ap().rearrange("(ic p) u -> p ic u", p=P))
                ris = psm.tile([P, NI], f32, tag="ris", name=f"ris{b}")
                nc.sync.dma_start(out=ris[:], in_=ri_in[b].ap())

                # ---- C2: psum [65, 512] = [i0|1].T @ R^T  (accum over items)
                P2 = pmm.tile([D + 1, CH], f32, tag="PC", name=f"P2_{b}")
                for ic in range(NI):
                    nc.tensor.matmul(P2[:], i0s[:, ic, :], At[:, ic, :],
                                     start=(ic == 0), stop=(ic == NI - 1))
                S2 = psm.tile([D + 1, CH], f32, tag="S2", name=f"S2_{b}")
                nc.vector.tensor_copy(out=S2[:], in_=P2[:])
                PT2 = ptr.tile([P, NU, D + 1], f32, tag="PT2", name=f"PT2_{b}")
                for uc in range(NU):
                    nc.tensor.transpose(PT2[:, uc, :],
                                        S2[:, uc * P:(uc + 1) * P],
                                        ident[0:D + 1, 0:D + 1])
                rut = psm.tile([P, NU, 1], f32, tag="rut", name=f"rut{b}")
                nc.vector.tensor_scalar_add(out=rut[:], in0=PT2[:, :, D:D + 1],
                                            scalar1=EPS)
                ru = psm.tile([P, NU, 1], f32, tag="ru", name=f"ru{b}")
                nc.vector.reciprocal(out=ru[:], in_=rut[:])
                u1f = psm.tile([P, NU, D], f32, tag="u1f", name=f"u1f{b}")
                for uc in range(NU):
                    nc.scalar.activation(out=u1f[:, uc, :], in_=PT2[:, uc, 0:D],
                                         func=CPY, scale=ru[:, uc, :])
                u1b = psm.tile([P, NU, D], bf16, tag="u1b", name=f"u1b{b}")
                nc.vector.tensor_copy(out=u1b[:], in_=u1f[:])
                L = psm.tile([P, NU, 2 * D], bf16, tag="L", name=f"L{b}")
                nc.vector.tensor_copy(out=L[:, :, 0:D], in_=u0s[:])
                nc.vector.tensor_copy(out=L[:, :, D:2 * D], in_=u1b[:])

                # ---- C1+C4 packed: [u0 | u1].T @ R-shard -> z.T | z2.T
                #      R-shard streamed from DRAM in [128, 4, 512] chunks
                zT = pone.tile([D, N], f32, tag="zT", name=f"zT{b}")
                wT = pone.tile([D, N], f32, tag="wT", name=f"wT{b}")
                for n in range(NCH):
                    Ac = pchunk.tile([P, NU, CH], bf16, tag="Ac",
                                     name=f"Ac{b}_{n}")
                    nc.sync.dma_start(
                        out=Ac[:],
                        in_=R_in[b].ap().rearrange(
                            "(uc p) n -> p uc n", p=P)[:, :,
                                                       n * CH:(n + 1) * CH])
                    P14 = pmm14.tile([P, CH], f32, tag="P14",
                                     name=f"P14_{b}_{n}")
                    for uc in range(NU):
                        nc.tensor.matmul(P14[:], L[:, uc, :], Ac[:, uc, :],
                                         start=(uc == 0), stop=(uc == NU - 1))
                    nc.vector.tensor_copy(out=zT[:, n * CH:(n + 1) * CH],
                                          in_=P14[0:D, :])
                    nc.vector.tensor_copy(out=wT[:, n * CH:(n + 1) * CH],
                                          in_=P14[D:2 * D, :])

                # ---- AllReduce z
                z_in = pdr.tile([D, N], f32, tag="z_in", name=f"z_in{b}")
                nc.scalar.dma_start(out=z_in[:], in_=zT[:])
                z_out = pdr.tile([D, N], f32, tag="z_out",
                                 name=f"z_out{b}", addr_space="Shared")
                nc.gpsimd.collective_compute(
                    "AllReduce", ALU.add, replica_groups=rg,
                    ins=[z_in.opt()], outs=[z_out.opt()])
                state[b] = (At, ris, ru, u1f, wT, z_out)

            def back(b):
                At, ris, ru, u1f, wT, z_out = state.pop(b)
                nc.scalar.dma_start(out=z_r_out[b].ap(), in_=z_out[:])

                # ---- i1 = z * ri in natural layout (cast, xbar transpose,
                #      per-item scale on the scalar engine)
                zb = pone.tile([D, N], bf16, tag="zb", name=f"zb{b}")
                nc.gpsimd.dma_start(out=zb[:], in_=z_out[:])
                i1u = pone.tile([P, NI, D], bf16, tag="i1u", name=f"i1u{b}")
                nc.sync.dma_start_transpose(out=i1u[:], in_=zb[:])
                i1b = pone.tile([P, NI, D], bf16, tag="i1b", name=f"i1b{b}")
                for ic in range(NI):
                    nc.scalar.activation(out=i1b[:, ic, :], in_=i1u[:, ic, :],
                                         func=CPY, scale=ris[:, ic:ic + 1])

                # ---- C3: u2_un.T = i1.T @ R^T
                P3 = pmm.tile([D, CH], f32, tag="PC", name=f"P3_{b}")
                for ic in range(NI):
                    nc.tensor.matmul(P3[:], i1b[:, ic, :], At[:, ic, :],
                                     start=(ic == 0), stop=(ic == NI - 1))
                S3 = psm.tile([D, CH], f32, tag="S3", name=f"S3_{b}")
                nc.vector.tensor_copy(out=S3[:], in_=P3[:])
                PT3 = ptr.tile([P, NU, D], f32, tag="PT3", name=f"PT3_{b}")
                for uc in range(NU):
                    nc.tensor.transpose(PT3[:, uc, :],
                                        S3[:, uc * P:(uc + 1) * P],
                                        ident[0:D, 0:D])
                u2f = psm.tile([P, NU, D], f32, tag="u2f", name=f"u2f{b}")
                for uc in range(NU):
                    nc.scalar.activation(out=u2f[:, uc, :], in_=PT3[:, uc, :],
                                         func=CPY, scale=ru[:, uc, :])
                uacc = psm.tile([P, NU, D], f32, tag="uacc", name=f"uacc{b}")
                nc.vector.tensor_add(out=uacc[:], in0=u2f[:], in1=u1f[:])

                nc.sync.dma_start(out=uacc_out[b].ap(), in_=uacc[:])
                nc.sync.dma_start(out=w_out[b].ap(), in_=wT[:])

            # software pipeline: emit back(b) after front(b+1) so the PE
            # stream never stalls on behavior b's AllReduce
            front(0)
            for b in range(1, nb):
                front(b)
                back(b - 1)
            back(nb - 1)

    nc.compile()
    return nc


# --------------------------------------------------------------------------
# host-side helpers
# --------------------------------------------------------------------------

def _swz_items(x):
    """[4096, C] -> [128, 32, C] with item = ic*128 + p."""
    return np.ascontiguousarray(x.reshape(NI, P, x.shape[1]).transpose(1, 0, 2))


def _swz_users(x):
    """[512, C] -> [128, 4, C] with user = uc*128 + p."""
    return np.ascontiguousarray(x.reshape(NU, P, x.shape[1]).transpose(1, 0, 2))


def prep_in_maps(dense_mats, u0, i0):
    """dense_mats: list of (R_bf16 [N,N], ri_nat [128, 32] f32)."""
    i0_aug = np.concatenate(
        [i0.astype(_BF16), np.ones((N, 1), _BF16)], axis=1)
    i0s = _swz_items(i0_aug)
    in_maps = []
    for k in range(NCORES):
        m = {"i0s": i0s,
             "u0s": _swz_users(u0[k * ULOC:(k + 1) * ULOC].astype(_BF16))}
        for b, (Rb, ri_nat) in enumerate(dense_mats):
            m[f"R{b}"] = np.ascontiguousarray(Rb[k * ULOC:(k + 1) * ULOC, :])
            m[f"Rt{b}"] = _per_core_rt(Rb, k)
            m[f"ri{b}"] = ri_nat
        in_maps.append(m)
    return in_maps


def host_prep_behavior(R):
    """Cast to bf16 + compute item-degree reciprocal (natural layout)."""
    Rb = R.astype(_BF16)
    deg = R.sum(axis=0, dtype=np.float64)
    ri_vec = (1.0 / (deg + EPS)).astype(np.float32)
    ri_nat = np.ascontiguousarray(ri_vec.reshape(NI, P).T)
    return Rb, ri_nat, deg.astype(np.float32)


def _per_core_rt(Rb, k):
    """Contiguous [N, ULOC] transposed shard for core k."""
    return np.ascontiguousarray(Rb[k * ULOC:(k + 1) * ULOC, :].T)


def assemble_dense(results, degs, nb):
    """Per-behavior (u_acc [N,D], i_acc [N,D]) from per-core outputs."""
    out = []
    for b in range(nb):
        u = np.concatenate(
            [results[k][f"uacc{b}"].transpose(1, 0, 2).reshape(ULOC, D)
             for k in range(NCORES)], axis=0) * np.float32(0.5)
        w = np.sum([results[k][f"wT{b}"] for k in range(NCORES)], axis=0,
                   dtype=np.float32)
        z = results[0][f"zr{b}"]
        i_acc = ((z + w) * np.float32(0.5)
                 / (degs[b] + np.float32(EPS))[None, :]).T
        out.append((np.ascontiguousarray(u, dtype=np.float32),
                    np.ascontiguousarray(i_acc, dtype=np.float32)))
    return out


def ones_behavior(u0, i0):
    """Analytic LightGCN-2-layer outputs when R is all-ones [N, N]."""
    s_i = i0.astype(np.float64).sum(axis=0)
    s_u = u0.astype(np.float64).sum(axis=0)
    d = N + EPS
    u_row = (s_i / d + s_u * N / (d * d)) * 0.5
    i_row = (s_u / d + s_i * N / (d * d)) * 0.5
    u = np.broadcast_to(u_row.astype(np.float32), (N, D)).copy()
    it = np.broadcast_to(i_row.astype(np.float32), (N, D)).copy()
    return u, it


# --------------------------------------------------------------------------
# cached device runner (compile once per behavior-count, run many)
# --------------------------------------------------------------------------

_RUNNERS = {}


class _Runner:
    def __init__(self, nb):
        self.nb = nb
        self.nc = build_program(nb)
        self._jitted = None
        self._meta = None

    def _prep_jit(self):
        import jax
        import numpy as _np
        from jax.sharding import Mesh, PartitionSpec
        from jax.experimental.shard_map import shard_map
        from concourse import bass2jax
        from concourse.bass2jax import _bass_exec_p, partition_id_tensor
        import concourse.mybir as mybir

        bass2jax.install_neuronx_cc_hook()
        nc = self.nc
        partition_name = (nc.partition_id_tensor.name
                          if nc.partition_id_tensor else None)
        in_names, out_names, out_avals, zero_shapes = [], [], [], []
        for alloc in nc.m.functions[0].allocations:
            if not isinstance(alloc, mybir.MemoryLocationSet):
                continue
            name = alloc.memorylocations[0].name
            if alloc.kind == "ExternalInput":
                if name != partition_name:
                    in_names.append(name)
            elif alloc.kind == "ExternalOutput":
                shape = tuple(alloc.tensor_shape)
                dtype = mybir.dt.np(alloc.dtype)
                out_names.append(name)
                out_avals.append(jax.core.ShapedArray(shape, dtype))
                zero_shapes.append((shape, dtype))
        n_params = len(in_names)
        full_in_names = list(in_names) + list(out_names)
        if partition_name is not None:
            full_in_names.append(partition_name)

        def _body(*args):
            operands = list(args)
            if partition_name is not None:
                operands.append(partition_id_tensor())
            outs = _bass_exec_p.bind(
                *operands,
                out_avals=tuple(out_avals),
                in_names=tuple(full_in_names),
                out_names=tuple(out_names),
                lowering_input_output_aliases=(),
                sim_require_finite=True,
                sim_require_nnan=True,
                nc=nc,
            )
            return tuple(outs)

        devices = jax.devices()[:NCORES]
        mesh = Mesh(_np.asarray(devices), ("core",))
        n_outs = len(out_names)
        in_specs = (PartitionSpec("core"),) * (n_params + n_outs)
        out_specs = (PartitionSpec("core"),) * n_outs
        donate = tuple(range(n_params, n_params + n_outs))
        self._jitted = jax.jit(
            shard_map(_body, mesh=mesh, in_specs=in_specs,
                      out_specs=out_specs, check_rep=False),
            donate_argnums=donate, keep_unused=True)
        self._meta = (in_names, out_names, out_avals, zero_shapes, n_params)

    def run(self, in_maps):
        if self._jitted is None:
            self._prep_jit()
        import numpy as _np
        in_names, out_names, out_avals, zero_shapes, n_params = self._meta
        concat_in = [
            _np.concatenate([_np.asarray(in_maps[c][nm]) for c in range(NCORES)],
                            axis=0)
            for nm in in_names]
        concat_zeros = [_np.zeros((NCORES * s[0], *s[1:]), dt)
                        for (s, dt) in zero_shapes]
        out_arrs = self._jitted(*concat_in, *concat_zeros)
        results = []
        for c in range(NCORES):
            results.append({
                nm: _np.asarray(out_arrs[i]).reshape(
                    NCORES, *out_avals[i].shape)[c]
                for i, nm in enumerate(out_names)})
        return results

    def run_traced(self, in_maps, tmpdir=None):
        """Run through run_bass_kernel_spmd with NTFF tracing (recompiles)."""
        _install_trace_shims()
        from concourse.bass_utils import run_bass_kernel_spmd
        return run_bass_kernel_spmd(self.nc, in_maps,
                                    core_ids=list(range(NCORES)),
                                    trace=True, tmpdir=tmpdir)


def _install_trace_shims():
    """This image's antenv lacks axon_hooks (the NTFF-hook registry) and has
    no artifact bucket; recreate the hook from the boot recipe and make
    artifact upload a local no-op."""
    import sys, types, importlib.util

    if "antenv.axon_hooks" not in sys.modules:
        mod = types.ModuleType("antenv.axon_hooks")
        mod._hook = None

        def set_axon_ntff_profile_hook(h):
            mod._hook = h

        def get_axon_ntff_profile_hook():
            return mod._hook

        mod.set_axon_ntff_profile_hook = set_axon_ntff_profile_hook
        mod.get_axon_ntff_profile_hook = get_axon_ntff_profile_hook
        import antenv
        sys.modules["antenv.axon_hooks"] = mod
        antenv.axon_hooks = mod

        spec = importlib.util.spec_from_file_location(
            "trn_boot_shim", "/root/.axon_site/trn_agent_boot/trn_boot.py")
        boot = importlib.util.module_from_spec(spec)
        spec.loader.exec_module(boot)
        hook = boot._ntff_profile_via_ctypes("/opt/axon/libaxon_pjrt.so")
        mod._hook = hook

    import concourse.bass_utils as bu
    if not getattr(bu.upload_artifacts, "_is_local_shim", False):
        def _local_upload(tmpdir):
            return tmpdir
        _local_upload._is_local_shim = True
        bu.upload_artifacts = _local_upload


def get_runner(nb):
    if nb not in _RUNNERS:
        _RUNNERS[nb] = _Runner(nb)
    return _RUNNERS[nb]


# --------------------------------------------------------------------------
# entry point
# --------------------------------------------------------------------------

def _is_ones(a):
    return a[0, 0] == 1.0 and bool(np.all(a == np.float32(1.0)))


def kernel(**inputs):
    inputs = {k: np.asarray(v) for k, v in inputs.items()}
    u0 = np.ascontiguousarray(inputs["user_embedding"], dtype=np.float32)
    i0 = np.ascontiguousarray(inputs["item_embedding"], dtype=np.float32)

    real_names = ["R_click", "R_fav", "R_cart", "R_buy"]
    virt_names = [("M_click", "add_click"), ("M_fav", "add_fav"),
                  ("M_cart", "add_cart")]
    mats = [np.asarray(inputs[n], dtype=np.float32) for n in real_names]
    mats += [np.asarray(inputs[m], dtype=np.float32) for m, _ in virt_names]

    dense_idx = [j for j, a in enumerate(mats) if not _is_ones(a)]
    per_behavior = [None] * 7

    if dense_idx:
        nb = len(dense_idx)
        runner = get_runner(nb)
        prepped = [host_prep_behavior(mats[j]) for j in dense_idx]
        in_maps = prep_in_maps([(p[0], p[1]) for p in prepped], u0, i0)
        results = runner.run(in_maps)
        dense = assemble_dense(results, [p[2] for p in prepped], nb)
        for pos, j in enumerate(dense_idx):
            per_behavior[j] = dense[pos]

    ones_cache = None
    for j, a in enumerate(mats):
        if per_behavior[j] is None:
            if ones_cache is None:
                ones_cache = ones_behavior(u0, i0)
            per_behavior[j] = ones_cache

    ur = [per_behavior[j][0] for j in range(4)]
    ir = [per_behavior[j][1] for j in range(4)]
    uv = [per_behavior[4 + j][0] + np.asarray(inputs[virt_names[j][1]],
                                              dtype=np.float32)
          for j in range(3)]
    iv = [per_behavior[4 + j][1] for j in range(3)]

    out = np.concatenate(
        [np.stack(ur), np.stack(ir), np.stack(uv), np.stack(iv)], axis=0)
    return np.ascontiguousarray(out, dtype=np.float32)
